# revision 36
# baseline (speedup 1.0000x reference)
"""2-layer GCN encoder on 8 Trainium2 NeuronCores (Bass/Tile).

Math: with dis = deg^{-1/2} (self-loops included), the GCN layer
    out = relu(D^{-1/2} A D^{-1/2} (X W) + b)
separates as
    out[v] = relu(dis[v] * (sum_{e: dst=v} dis[src]*X[src]) @ W + b)
so the per-edge norm disappears and both weight matmuls commute out of the
edge aggregation.  Aggregation is done as binary-selection matmuls on the
TensorEngine over dma_gather'ed rows of the dis-prescaled feature table.

Design (v2):
- The feature table is stored in gid order for BOTH layers, so the two
  layers share identical dst-slot streams (the S3 one-hot build and the dl
  stream are the same data) and identical cell structure.
- The added self-loops never enter the gather: each group's own rows are a
  contiguous static DMA and enter the PSUM accumulation through one extra
  matmul against a constant identity tile.
- chunk(edge) = slot(src) % 4 (int16 gather index constraint).  A greedy
  class-balancing pass assigns slot%4 classes to nodes to equalize the
  (dst-bin, chunk) cell sizes, so nearly all cells fit 4 tiles of 128.
- Tiles per group are a data-fitted static profile (compiled after seeing
  the graph): per-core bins are rank-matched to profile slots.
- Epilogue runs on the otherwise-idle Scalar engine when biases are zero:
  gshard = relu(dis^2 * po) and out = dis * po are exact single activations.
- The inter-layer AllGather is split into 4 chunks (separate DRAM tiles)
  so the collective streams while layer 1 is still computing.
"""

import numpy as np
import ml_dtypes

import concourse.bacc as bacc
import concourse.tile as tile
import concourse.mybir as mybir
import concourse.bass as bass
from concourse.bass_utils import run_bass_kernel_spmd

# problem shapes (hardcoded per contract)
N = 100000
E = 1600000
IN_DIM, HID, OUT_DIM = 128, 128, 64

# schedule constants
P = 128           # partitions / tile edge count
NC_ = 8           # cores
G = 102           # groups per core
W = 6             # groups per batch
NB = 17           # batches per layer (W*NB == G)
NSEC = 4          # chunks (slot(src) mod 4)
NODES_PC = G * P  # padded nodes per core       = 13056
GFULL = NC_ * NODES_PC  # gathered table rows   = 104448
NBINS = NC_ * G
# AllGather chunk boundaries in group-rank space (multiples of W)
CC_BOUNDS = [0, 30, 60, 90, 102]
NCC = len(CC_BOUNDS) - 1

BF16 = ml_dtypes.bfloat16

_compiled = {}  # cache across calls, keyed by (T_profile, bias_free)


# ----------------------------------------------------------------- host side

def _pack_nodes(deg):
    """Bin-pack nodes into 8*G bins (<=128 nodes each), balancing in-degree.

    Returns bin_of [N] and core_of_bin [NBINS] (bins snake-assigned to cores
    by load so core totals balance).
    """
    import heapq
    order = np.argsort(-deg, kind="stable")
    counts = np.zeros(NBINS, np.int64)
    loads = np.zeros(NBINS, np.float64)
    bin_of = np.empty(N, np.int64)
    h = [(0.0, b) for b in range(NBINS)]
    heapq.heapify(h)
    for n in order:
        while True:
            load, b = heapq.heappop(h)
            if counts[b] < P:
                break
        bin_of[n] = b
        counts[b] += 1
        loads[b] = load + deg[n]
        if counts[b] < P:
            heapq.heappush(h, (loads[b], b))
    bins_sorted = np.argsort(-loads, kind="stable")
    core_of_bin = np.empty(NBINS, np.int64)
    for r, b in enumerate(bins_sorted):
        rnd, pos = divmod(r, NC_)
        core_of_bin[b] = pos if rnd % 2 == 0 else NC_ - 1 - pos
    return bin_of, core_of_bin


def _balance_classes(src, dstbin, bin_of, out_deg_order):
    """Assign each node a slot%4 class to balance per-(dstbin, class) cell
    sizes, respecting <=32 nodes per class per source bin."""
    cls = np.full(N, -1, np.int8)
    M = np.zeros((NBINS, NSEC), np.float64)   # cell loads
    cap = np.zeros((NBINS, NSEC), np.int32)   # class occupancy per src bin
    # per-node CSR over dst bins
    order = np.argsort(src, kind="stable")
    s_sorted = src[order]
    b_sorted = dstbin[order]
    starts = np.searchsorted(s_sorted, np.arange(N))
    ends = np.searchsorted(s_sorted, np.arange(N) + 1)
    rr = 0
    for v in out_deg_order:
        sb = bin_of[v]
        lo, hi = starts[v], ends[v]
        capv = cap[sb]
        if hi > lo:
            bins_v, k_v = np.unique(b_sorted[lo:hi], return_counts=True)
            scores = k_v @ M[bins_v]
            scores = np.where(capv < 32, scores, np.inf)
            r = int(np.argmin(scores))
            M[bins_v, r] += k_v
        else:
            # no out-edges: round-robin into free classes
            free = np.flatnonzero(capv < 32)
            r = int(free[rr % len(free)])
            rr += 1
        cls[v] = r
        cap[sb, r] += 1
    return cls


def _build_layout(T_prof):
    """Static tile layout from the per-group-rank tile profile.

    Returns dict with per-batch tile offsets and per-(b,c,j) tile bases.
    """
    secT = []          # tiles per chunk-section per batch
    bt = []            # tiles per batch
    b_off = []         # global tile offset of batch start
    jb_off = []        # [NB][W] within-section tile base for group rank j
    off = 0
    for b in range(NB):
        Tg = [int(T_prof[b * W + j]) for j in range(W)]
        cum = np.concatenate([[0], np.cumsum(Tg)]).astype(int)
        secT.append(int(cum[-1]))
        bt.append(int(cum[-1]) * NSEC)
        b_off.append(off)
        jb_off.append([int(c) for c in cum[:-1]])
        off += int(cum[-1]) * NSEC
    return dict(secT=secT, bt=bt, b_off=b_off, jb_off=jb_off, total=off)


def _build_schedule(src_row, cellrank, eslot, edge_cell_rankpos, lay):
    """Build per-core gather-index and dst-slot streams.

    src_row: per-edge row in the gather table (determines idx and chunk).
    cellrank: per-edge (core, grank, chunk) flattened cell id.
    eslot: per-edge dst slot.
    edge_cell_rankpos: rank of the edge within its cell (canonical order).
    """
    total = lay["total"]
    flat_idx = np.zeros(NC_ * total * P, np.int16)
    flat_dl = np.full(NC_ * total * P, P, np.int16)

    ch = cellrank % NSEC
    gg = (cellrank // NSEC) % G
    cr = cellrank // (NSEC * G)
    batch = gg // W
    j = gg % W
    b_off = np.asarray(lay["b_off"])[batch]
    secT = np.asarray(lay["secT"])[batch]
    jb = np.asarray([[lay["jb_off"][b][jj] for jj in range(W)] for b in range(NB)])
    tile_in_cell = edge_cell_rankpos // P
    pos = edge_cell_rankpos % P
    T = b_off + ch * secT + jb[batch, j] + tile_in_cell
    goff = cr * (total * P) + T * P + pos
    flat_idx[goff] = (src_row // NSEC).astype(np.int16)
    flat_dl[goff] = eslot.astype(np.int16)

    # wrapped idx layout per batch: wrapped[p, s] = flat[s*16 + p%16], x8
    idx_blocks = []
    dl_blocks = []
    fi = flat_idx.reshape(NC_, total, P)
    fd = flat_dl.reshape(NC_, total, P)
    for b in range(NB):
        o, n = lay["b_off"][b], lay["bt"][b]
        blk = fi[:, o:o + n, :].reshape(NC_, n * P // 16, 16)
        A = blk.transpose(0, 2, 1)  # [NC, 16, n*8]
        idx_blocks.append(np.tile(A, (1, 8, 1)))
        dl_blocks.append(fd[:, o:o + n, :].transpose(0, 2, 1))
    idx_dram = np.concatenate(idx_blocks, axis=2)
    dl_dram = np.concatenate(dl_blocks, axis=2).astype(BF16)
    return np.ascontiguousarray(idx_dram), np.ascontiguousarray(dl_dram)


def preprocess(x, edge_index):
    src = np.asarray(edge_index[0], dtype=np.int64)
    dst = np.asarray(edge_index[1], dtype=np.int64)
    deg = np.bincount(dst, minlength=N).astype(np.float64) + 1.0  # +self
    dis = (1.0 / np.sqrt(deg)).astype(np.float32)

    bin_of, core_of_bin = _pack_nodes(deg - 1.0)

    # class balancing: chunk(edge) = class(src)
    out_deg = np.bincount(src, minlength=N)
    out_order = np.argsort(-out_deg, kind="stable")
    cls = _balance_classes(src, bin_of[dst], bin_of, out_order)

    # slots: class-interleaved (slot % 4 == class)
    key = bin_of * 4 + cls
    order = np.argsort(key, kind="stable")
    ks = key[order]
    rank_in = np.arange(N) - np.searchsorted(ks, ks)
    slot_of = np.empty(N, np.int64)
    slot_of[order] = cls[order] + 4 * rank_in
    assert slot_of.max() < P

    # per-(core) bin -> gabs rank by descending tile need
    cell_cnt = np.zeros((NBINS, NSEC), np.int64)
    np.add.at(cell_cnt, (bin_of[dst], cls[src]), 1)
    need = np.maximum((cell_cnt.max(axis=1) + P - 1) // P, 1)

    gabs_of_bin = np.empty(NBINS, np.int64)
    T_prof = np.zeros(G, np.int64)
    for c in range(NC_):
        bins_c = np.flatnonzero(core_of_bin == c)
        assert len(bins_c) == G
        o = bins_c[np.argsort(-need[bins_c], kind="stable")]
        gabs_of_bin[o] = np.arange(G)
        T_prof = np.maximum(T_prof, need[o])

    node_core = core_of_bin[bin_of]
    node_gabs = gabs_of_bin[bin_of]
    gid = node_core * NODES_PC + node_gabs * P + slot_of

    lay = _build_layout(T_prof)

    # feature table in gid order (prescaled)
    xt = np.zeros((GFULL, IN_DIM), BF16)
    xt[gid] = (np.asarray(x, np.float32) * dis[:, None]).astype(BF16)

    # layer-2 table rows under the chunked-AllGather layout
    cc_row_base = np.zeros(NCC, np.int64)
    acc = 0
    for k in range(NCC):
        cc_row_base[k] = acc
        acc += NC_ * (CC_BOUNDS[k + 1] - CC_BOUNDS[k]) * P
    k_of_g = np.searchsorted(np.asarray(CC_BOUNDS[1:]), np.arange(G), side="right")
    ch_rows = np.asarray([(CC_BOUNDS[k + 1] - CC_BOUNDS[k]) * P for k in range(NCC)])
    kk = k_of_g[node_gabs]
    g2row = (cc_row_base[kk] + node_core * ch_rows[kk]
             + (node_gabs - np.asarray(CC_BOUNDS)[kk]) * P + slot_of)
    assert (g2row % NSEC == gid % NSEC).all()

    # canonical edge order: stable sort by cell
    grank = node_gabs  # already the rank
    ecell = ((node_core[dst] * G + grank[dst]) * NSEC + cls[src])
    eorder = np.argsort(ecell, kind="stable")
    cell_sorted = ecell[eorder]
    counts = np.bincount(ecell, minlength=NBINS * NSEC)
    cap = np.broadcast_to(T_prof[None, :, None], (NC_, G, NSEC)).reshape(-1) * P
    bad = counts > cap
    assert not bad.any(), f"profile overflow: {counts[bad].max()} vs {cap[bad].min()}"
    starts = np.concatenate([[0], np.cumsum(counts)[:-1]])
    rankpos = np.arange(len(cell_sorted)) - np.repeat(starts, counts)

    src_o, dst_o = src[eorder], dst[eorder]
    idx1, dl1 = _build_schedule(gid[src_o], cell_sorted, slot_of[dst_o], rankpos, lay)
    idx2, _ = _build_schedule(g2row[src_o], cell_sorted, slot_of[dst_o], rankpos, lay)

    dis_sb = np.zeros((NC_, P, G), np.float32)
    dis_sb[node_core, slot_of, node_gabs] = dis
    dis2_sb = dis_sb * dis_sb

    return dict(
        xt=xt, idx1=idx1, idx2=idx2, dl=dl1, dis_sb=dis_sb, dis2_sb=dis2_sb,
        gid=gid, lay=lay, T_prof=tuple(int(t) for t in T_prof),
    )


# --------------------------------------------------------------- device side

def build_program(T_prof, bias_free):
    f32 = mybir.dt.float32
    bf16 = mybir.dt.bfloat16
    i16 = mybir.dt.int16
    AO = mybir.AluOpType
    AF = mybir.ActivationFunctionType

    lay = _build_layout(np.asarray(T_prof))
    total = lay["total"]
    BTmax = max(lay["bt"])
    IDXW_TOT = 8 * total

    nc = bacc.Bacc("TRN2", target_bir_lowering=False, debug=False,
                   num_devices=NC_, num_swdge_queues=4)
    xt_d = nc.dram_tensor("xt", [GFULL, IN_DIM], bf16, kind="ExternalInput")
    xself_d = nc.dram_tensor("xself", [P, G * IN_DIM], bf16, kind="ExternalInput")
    idx1_d = nc.dram_tensor("idx1", [P, IDXW_TOT], i16, kind="ExternalInput")
    idx2_d = nc.dram_tensor("idx2", [P, IDXW_TOT], i16, kind="ExternalInput")
    dl_d = nc.dram_tensor("dl", [P, total], bf16, kind="ExternalInput")
    dis_d = nc.dram_tensor("dis", [P, G], f32, kind="ExternalInput")
    dis2_d = nc.dram_tensor("dis2", [P, G], f32, kind="ExternalInput")
    w1_d = nc.dram_tensor("w1", [IN_DIM, HID], bf16, kind="ExternalInput")
    w2_d = nc.dram_tensor("w2", [HID, OUT_DIM], bf16, kind="ExternalInput")
    b1_d = nc.dram_tensor("b1r", [P, HID], f32, kind="ExternalInput")
    b2_d = nc.dram_tensor("b2r", [P, OUT_DIM], f32, kind="ExternalInput")
    iota_d = nc.dram_tensor("iota", [P, P], bf16, kind="ExternalInput")
    ident_d = nc.dram_tensor("ident", [P, P], bf16, kind="ExternalInput")
    out_d = nc.dram_tensor("out", [NODES_PC, OUT_DIM], f32, kind="ExternalOutput")

    with tile.TileContext(nc) as tc:
        with tc.tile_pool(name="const", bufs=1) as cpool, \
             tc.tile_pool(name="io", bufs=3) as iopool, \
             tc.tile_pool(name="selfp", bufs=3) as selfpool, \
             tc.tile_pool(name="msgp", bufs=3) as mpool, \
             tc.tile_pool(name="sp", bufs=2) as spool, \
             tc.tile_pool(name="epi", bufs=4) as epool, \
             tc.tile_pool(name="aggp", bufs=8) as aggp, \
             tc.tile_pool(name="psag", bufs=6, space="PSUM") as psag, \
             tc.tile_pool(name="psep", bufs=2, space="PSUM") as psep, \
             tc.tile_pool(name="dram", bufs=1, space="DRAM") as dpool:

            w1s = cpool.tile([IN_DIM, HID], bf16)
            nc.sync.dma_start(out=w1s[:], in_=w1_d[:])
            w2s = cpool.tile([HID, OUT_DIM], bf16)
            nc.sync.dma_start(out=w2s[:], in_=w2_d[:])
            dis_s = cpool.tile([P, G], f32)
            nc.sync.dma_start(out=dis_s[:], in_=dis_d[:])
            dis2_s = cpool.tile([P, G], f32)
            nc.sync.dma_start(out=dis2_s[:], in_=dis2_d[:])
            iota_s = cpool.tile([P, P], bf16)
            nc.sync.dma_start(out=iota_s[:], in_=iota_d[:])
            ident_s = cpool.tile([P, P], bf16)
            nc.sync.dma_start(out=ident_s[:], in_=ident_d[:])
            dl_s = cpool.tile([P, total], bf16)
            nc.sync.dma_start(out=dl_s[:], in_=dl_d[:])
            selfh = cpool.tile([P, G * P], bf16)  # layer-2 self rows (dis*h)
            if not bias_free:
                b1s = cpool.tile([P, HID], f32)
                nc.sync.dma_start(out=b1s[:], in_=b1_d[:])
                b2s = cpool.tile([P, OUT_DIM], f32)
                nc.sync.dma_start(out=b2s[:], in_=b2_d[:])

            gsh = []
            for k in range(NCC):
                rows = (CC_BOUNDS[k + 1] - CC_BOUNDS[k]) * P
                gsh.append(dpool.tile([rows, HID], bf16, tag=f"gsh{k}",
                                      name=f"gsh{k}"))
            gfull = dpool.tile([GFULL, HID], bf16)

            xt_v = xt_d[:].rearrange("(n f) d -> n f d", f=NSEC)
            gf_v = gfull.rearrange("(n f) d -> n f d", f=NSEC)

            # group rank -> (cc chunk, row base within chunk tile)
            def gsh_loc(gabs):
                k = 0
                while CC_BOUNDS[k + 1] <= gabs:
                    k += 1
                return k, (gabs - CC_BOUNDS[k]) * P

            def sink1(gabs, po):
                gt = selfh[:, gabs * P:(gabs + 1) * P]
                if bias_free:
                    nc.scalar.activation(
                        out=gt, in_=po[:], func=AF.Relu,
                        scale=dis2_s[:, gabs:gabs + 1],
                    )
                else:
                    v = epool.tile([P, HID], mybir.dt.float32, tag="v")
                    nc.vector.scalar_tensor_tensor(
                        out=v[:], in0=po[:], scalar=dis_s[:, gabs:gabs + 1],
                        in1=b1s[:], op0=mybir.AluOpType.mult,
                        op1=mybir.AluOpType.add,
                    )
                    nc.vector.tensor_scalar(
                        out=gt, in0=v[:], scalar1=0.0,
                        scalar2=dis_s[:, gabs:gabs + 1],
                        op0=mybir.AluOpType.max, op1=mybir.AluOpType.mult,
                    )
                k, ro = gsh_loc(gabs)
                nc.sync.dma_start(out=gsh[k][ro:ro + P, :], in_=gt)

            def sink2(gabs, po):
                o = epool.tile([P, OUT_DIM], mybir.dt.float32, tag="o")
                if bias_free:
                    nc.scalar.activation(
                        out=o[:], in_=po[:], func=AF.Copy,
                        scale=dis_s[:, gabs:gabs + 1],
                    )
                else:
                    nc.vector.scalar_tensor_tensor(
                        out=o[:], in0=po[:], scalar=dis_s[:, gabs:gabs + 1],
                        in1=b2s[:], op0=mybir.AluOpType.mult,
                        op1=mybir.AluOpType.add,
                    )
                nc.sync.dma_start(out=out_d[gabs * P:(gabs + 1) * P, :], in_=o[:])

            def self1(b):
                return xself_d[:, b * W * P:(b + 1) * W * P].rearrange(
                    "p (w d) -> p w d", w=W)

            def layer1_with_cc():
                cc_rows = np.zeros(NCC + 1, np.int64)
                for k in range(NCC):
                    cc_rows[k + 1] = cc_rows[k] + NC_ * (CC_BOUNDS[k + 1] - CC_BOUNDS[k]) * P
                done_k = 0
                for b in range(NB):
                    layer_batch1(b)
                    while done_k < NCC and CC_BOUNDS[done_k + 1] <= (b + 1) * W:
                        emit_cc(done_k, cc_rows)
                        done_k += 1
                while done_k < NCC:
                    emit_cc(done_k, cc_rows)
                    done_k += 1

            def emit_cc(k, cc_rows):
                nc.gpsimd.collective_compute(
                    "AllGather",
                    mybir.AluOpType.bypass,
                    replica_groups=[list(range(NC_))],
                    ins=[gsh[k].opt()],
                    outs=[gfull[cc_rows[k]:cc_rows[k + 1], :].opt()],
                )

            # wrap single-batch emission of layer1 so CC can interleave
            def layer_batch1(b):
                _layer_one_batch(idx1_d, xt_v, self1, w1s, HID, sink1, b)

            def _layer_one_batch(idx_d, tbl_view, self_src, wsb, dout, sink, b):
                bt_b = lay["bt"][b]
                secT = lay["secT"][b]
                idxo = 8 * lay["b_off"][b]
                idx_t = iopool.tile([P, 8 * BTmax], mybir.dt.int16, tag="idx")
                nc.sync.dma_start(
                    out=idx_t[:, :8 * bt_b],
                    in_=idx_d[:, idxo:idxo + 8 * bt_b],
                )
                if self_src is not None:
                    selfx = selfpool.tile([P, W, P], mybir.dt.bfloat16, tag="selfx")
                    nc.sync.dma_start(out=selfx[:], in_=self_src(b))
                    self_ap = lambda j: selfx[:, j, :]
                else:
                    self_ap = lambda j: selfh[:, (b * W + j) * P:(b * W + j + 1) * P]
                msg = mpool.tile([P, BTmax, P], mybir.dt.bfloat16, tag="msg")
                for c in range(NSEC):
                    nc.gpsimd.dma_gather(
                        out_ap=msg[:, c * secT:(c + 1) * secT, :],
                        in_ap=tbl_view[:, c, :],
                        idxs_ap=idx_t[:, c * secT * 8:(c + 1) * secT * 8],
                        num_idxs=secT * P,
                        num_idxs_reg=secT * P,
                        elem_size=IN_DIM,
                        elem_step=IN_DIM * NSEC,
                        single_packet=False,
                        queue_num=c,
                    )
                S3 = spool.tile([P, BTmax, P], mybir.dt.bfloat16, tag="S3")
                do = lay["b_off"][b]
                sp = 0
                nc.vector.tensor_tensor(
                    out=S3[:, sp:bt_b, :],
                    in0=dl_s[:, do + sp:do + bt_b].unsqueeze(2)
                        .to_broadcast([P, bt_b - sp, P]),
                    in1=iota_s[:].unsqueeze(1).to_broadcast([P, bt_b - sp, P]),
                    op=mybir.AluOpType.is_equal,
                )
                aggTs = []
                for j in range(W):
                    gabs = b * W + j
                    Tg = T_prof[gabs]
                    ps = psag.tile([P, P], mybir.dt.float32, tag="agg")
                    nc.tensor.matmul(
                        out=ps[:], lhsT=self_ap(j), rhs=ident_s[:],
                        start=True, stop=False,
                    )
                    for c in range(NSEC):
                        for k in range(Tg):
                            t = c * secT + lay["jb_off"][b][j] + k
                            nc.tensor.matmul(
                                out=ps[:], lhsT=msg[:, t, :], rhs=S3[:, t, :],
                                start=False,
                                stop=(c == NSEC - 1 and k == Tg - 1),
                            )
                    aggT = aggp.tile([P, P], mybir.dt.bfloat16, tag="aggT")
                    nc.scalar.activation(
                        out=aggT[:], in_=ps[:],
                        func=mybir.ActivationFunctionType.Copy)
                    aggTs.append((gabs, aggT))
                for gabs, aggT in aggTs:
                    po = psep.tile([P, dout], mybir.dt.float32, tag="po")
                    nc.tensor.matmul(
                        out=po[:], lhsT=aggT[:], rhs=wsb[:], start=True, stop=True,
                    )
                    sink(gabs, po)

            layer1_with_cc()

            for b in range(NB):
                _layer_one_batch(idx2_d, gf_v, None, w2s, OUT_DIM, sink2, b)

    nc.compile()
    return nc


# ------------------------------------------------------------------- runner

def run(inputs, trace=False):
    global _compiled
    x = np.asarray(inputs["x"], np.float32)
    edge_index = np.asarray(inputs["edge_index"])
    W1 = np.asarray(inputs["W1"], np.float32)
    b1 = np.asarray(inputs["b1"], np.float32)
    W2 = np.asarray(inputs["W2"], np.float32)
    b2 = np.asarray(inputs["b2"], np.float32)

    pp = preprocess(x, edge_index)
    bias_free = (np.abs(b1).max() == 0.0) and (np.abs(b2).max() == 0.0)
    key = (pp["T_prof"], bias_free)
    if key not in _compiled:
        _compiled.clear()
        _compiled[key] = build_program(pp["T_prof"], bias_free)
    nc = _compiled[key]

    iota = np.broadcast_to(np.arange(P, dtype=np.float32), (P, P)).astype(BF16)
    ident = np.eye(P, dtype=np.float32).astype(BF16)
    b1r = np.broadcast_to(b1, (P, HID)).astype(np.float32)
    b2r = np.broadcast_to(b2, (P, OUT_DIM)).astype(np.float32)
    w1b = W1.astype(BF16)
    w2b = W2.astype(BF16)

    in_maps = []
    for c in range(NC_):
        in_maps.append({
            "xt": pp["xt"],
            "xself": np.ascontiguousarray(
                pp["xt"][c * NODES_PC:(c + 1) * NODES_PC]
                .reshape(G, P, IN_DIM).transpose(1, 0, 2).reshape(P, G * IN_DIM)),
            "idx1": pp["idx1"][c],
            "idx2": pp["idx2"][c],
            "dl": pp["dl"][c],
            "dis": pp["dis_sb"][c],
            "dis2": pp["dis2_sb"][c],
            "w1": w1b,
            "w2": w2b,
            "b1r": np.ascontiguousarray(b1r),
            "b2r": np.ascontiguousarray(b2r),
            "iota": np.ascontiguousarray(iota),
            "ident": np.ascontiguousarray(ident),
        })

    res = run_bass_kernel_spmd(
        nc, in_maps, core_ids=list(range(NC_)), trace=trace
    )
    allf = np.concatenate([res.results[c]["out"] for c in range(NC_)], axis=0)
    out = allf[pp["gid"]].astype(np.float32)
    return out, res


def kernel(**inputs):
    out, _ = run(inputs, trace=False)
    return out
